# revision 1
# baseline (speedup 1.0000x reference)
"""Trainium2 Bass kernel for nn_LocalizedFiltering (fused cat-conv2d x2 + residual + RMSNorm).

Strategy: sequence-parallel across 8 NeuronCores (one sequence of 2048 tokens +
1 cache row per core) -- no collectives needed. On-device compute uses a
transposed (feature-on-partition) layout so the kernel-2 causal conv's
shift-add becomes a column-window offset absorbed into the matmul rhs windows.
Matmuls run in bf16 (fp32 PSUM accumulation); residual + RMSNorm in fp32.

Per core s:
  xt1T = [cache1_s ; X_s].T                     # [2048, 2049] bf16
  layer1: c = xt1T.T@W1 windows -> o1T          # [1024, 2048] -> xt2T cols 1..
  layer2: same with W2 -> o2T tiles [128,512]
  epilogue: PE-transpose to row-major, + X residual, RMSNorm, DMA out rows.
ln_weight is applied exactly on the host (out *= ln_weight).
"""

import os

import numpy as np
import ml_dtypes

BS, L, D, CACHE = 8, 2048, 2048, 64
T = BS * L
H = D // 2          # 1024
EPS = 1e-6
NCORES = 8
BLK = 512           # token block (= one PSUM bank of fp32)
NBLK = L // BLK     # 4
KT1 = D // 128      # 16 contraction tiles, layer 1
KT2 = H // 128      # 8 contraction tiles, layer 2
QT1 = H // 128      # 8 output-feature tiles, layer 1 (per half)
QT2 = D // 128      # 16 output-feature tiles, layer 2 (per half)

TRACE = bool(int(os.environ.get("BASS_KERNEL_TRACE", "0")))
LAST_EXEC_NS = None
LAST_RESULTS = None

_NC_CACHE = {}


def _build_bass():
    if "nc" in _NC_CACHE:
        return _NC_CACHE["nc"]

    import concourse.bacc as bacc
    import concourse.tile as tile
    import concourse.mybir as mybir
    from concourse.masks import make_identity

    fp32 = mybir.dt.float32
    bf16 = mybir.dt.bfloat16
    Act = mybir.ActivationFunctionType

    nc = bacc.Bacc("TRN2", target_bir_lowering=False)

    xt1 = nc.declare_dram_parameter("xt1", [D, L + 1], bf16, isOutput=False)
    xrow = nc.declare_dram_parameter("xrow", [L, D], fp32, isOutput=False)
    c2 = nc.declare_dram_parameter("c2", [H, 1], bf16, isOutput=False)
    w1 = nc.declare_dram_parameter("w1", [D, D], bf16, isOutput=False)
    w2 = nc.declare_dram_parameter("w2", [H, 2 * D], bf16, isOutput=False)
    b1 = nc.declare_dram_parameter("b1", [H, 1], fp32, isOutput=False)
    b2 = nc.declare_dram_parameter("b2", [D, 1], fp32, isOutput=False)
    out = nc.declare_dram_parameter("out", [L, D], fp32, isOutput=True)

    with tile.TileContext(nc) as tc, \
            tc.tile_pool(name="wpool", bufs=1) as wpool, \
            tc.tile_pool(name="wpre", bufs=1) as wpre, \
            tc.tile_pool(name="xt1p", bufs=2) as xt1p, \
            tc.tile_pool(name="xt2p", bufs=1) as xt2p, \
            tc.tile_pool(name="xrowp", bufs=8) as xrowp, \
            tc.tile_pool(name="rowp", bufs=4) as rowp, \
            tc.tile_pool(name="tmp", bufs=2) as tmp, \
            tc.tile_pool(name="const", bufs=1) as const, \
            tc.tile_pool(name="psmm", bufs=3, space="PSUM") as psmm, \
            tc.tile_pool(name="psdp", bufs=1, space="PSUM") as psdp, \
            tc.tile_pool(name="pstr", bufs=4, space="PSUM") as pstr:

        ident = const.tile([128, 128], fp32)
        make_identity(nc, ident)
        epssb = const.tile([128, 1], fp32)
        nc.vector.memset(epssb, EPS)

        b1sb = const.tile([128, QT1, 1], fp32)
        b2sb = const.tile([128, QT2, 1], fp32)
        xt2sb = xt2p.tile([128, KT2, L + 1], bf16)

        # ---------------- Phase A: layer 1 -> xt2T (bf16) ----------------
        # W1 as 8 pair-tiles [128, 2, D]; same slots later reused by W2 k-tiles.
        NW = KT1 // 2  # 8
        w1t = []
        x1k0 = []
        # interleave issue order: w pair j, then 2 x1 k-tiles of block 0, so the
        # first matmuls unblock after a few MB of DMA.
        for j in range(NW):
            wj = wpool.tile([128, 2, D], bf16, tag=f"w{j}", name=f"w1_{j}")
            for kk in range(2):
                nc.sync.dma_start(
                    out=wj[:, kk, :],
                    in_=w1[(2 * j + kk) * 128:(2 * j + kk + 1) * 128, :])
            w1t.append(wj)
            for kk in range(2):
                k = 2 * j + kk
                xk = xt1p.tile([128, BLK + 1], bf16, tag=f"x1k{k}", name=f"x1_0_{k}")
                nc.sync.dma_start(
                    out=xk, in_=xt1[k * 128:(k + 1) * 128, 0:BLK + 1])
                x1k0.append(xk)

        nc.sync.dma_start(
            out=b1sb, in_=b1.rearrange("(q p) o -> p q o", p=128))
        nc.sync.dma_start(
            out=b2sb, in_=b2.rearrange("(q p) o -> p q o", p=128))
        nc.sync.dma_start(
            out=xt2sb[:, :, 0:1], in_=c2.rearrange("(k p) o -> p k o", p=128))

        for b in range(NBLK):
            if b == 0:
                x1k = x1k0
            else:
                x1k = []
                for k in range(KT1):
                    xk = xt1p.tile([128, BLK + 1], bf16, tag=f"x1k{k}",
                                   name=f"x1_{b}_{k}")
                    nc.sync.dma_start(
                        out=xk,
                        in_=xt1[k * 128:(k + 1) * 128, b * BLK:b * BLK + BLK + 1])
                    x1k.append(xk)
            # k-outer over 8 concurrent psum groups: PE consumes each W1 pair
            # as it lands (startup), and frees W1 slots progressively on the
            # last block so the W2 stream overlaps the tail of phase A.
            psA = []
            for q in range(QT1):
                if q < 3:
                    ps = psmm.tile([128, BLK], fp32, tag="mm", name=f"psA_{b}_{q}")
                elif q < 7:
                    ps = pstr.tile([128, BLK], fp32, tag="pt", name=f"psA_{b}_{q}")
                else:
                    ps = psdp.tile([128, BLK], fp32, tag="dump", name=f"psA_{b}_{q}")
                psA.append(ps)
            for k in range(KT1):
                for q in range(QT1):
                    nc.tensor.matmul(
                        psA[q], lhsT=w1t[k // 2][:, k % 2, q * 128:(q + 1) * 128],
                        rhs=x1k[k][:, 0:BLK],
                        start=(k == 0), stop=False)
                    nc.tensor.matmul(
                        psA[q],
                        lhsT=w1t[k // 2][:, k % 2, H + q * 128:H + (q + 1) * 128],
                        rhs=x1k[k][:, 1:BLK + 1],
                        start=False, stop=(k == KT1 - 1))
            for q in range(QT1):
                nc.scalar.activation(
                    out=xt2sb[:, q, 1 + b * BLK:1 + (b + 1) * BLK], in_=psA[q],
                    func=Act.Identity, bias=b1sb[:, q, :], scale=1.0)

        # ---------------- Phase B: layer 2 + residual + RMSNorm ----------------
        # W2 k-tiles: k=0,1 prefetched into dedicated slots; k>=2 reuse w slots.
        w2t = []
        for k in range(KT2):
            if k < 2:
                wk = wpre.tile([128, 2 * D], bf16, tag=f"wp{k}", name=f"w2_{k}")
            else:
                wk = wpool.tile([128, 2 * D], bf16, tag=f"w{k - 2}", name=f"w2_{k}")
            nc.sync.dma_start(out=wk, in_=w2[k * 128:(k + 1) * 128, :])
            w2t.append(wk)

        for b in range(NBLK):
            rowcs = []
            accs = []
            xrcs = {}

            def load_xr_group(c4, b=b, xrcs=xrcs):
                sl = slice(c4 * BLK, (c4 + 1) * BLK)
                for c in range(4):
                    t = xrowp.tile([128, BLK], fp32, tag="xrc",
                                   name=f"xrc_{b}_{c}_{c4}")
                    r0 = b * BLK + c * 128
                    nc.sync.dma_start(out=t, in_=xrow[r0:r0 + 128, sl])
                    xrcs[(c, c4)] = t

            for c in range(4):
                rowcs.append(rowp.tile([128, D], fp32, tag="rowc", name=f"rowc_{b}_{c}"))
                accs.append(tmp.tile([128, 4], fp32, tag=f"acc4_{c}",
                                     name=f"acc4_{b}_{c}"))
            load_xr_group(0)
            for q in range(QT2):
                ps = psmm.tile([128, BLK], fp32, tag="mm", name=f"psB_{b}_{q}")
                for k in range(KT2):
                    nc.tensor.matmul(
                        ps, lhsT=w2t[k][:, q * 128:(q + 1) * 128],
                        rhs=xt2sb[:, k, b * BLK:(b + 1) * BLK],
                        start=(k == 0), stop=False)
                for k in range(KT2):
                    nc.tensor.matmul(
                        ps, lhsT=w2t[k][:, D + q * 128:D + (q + 1) * 128],
                        rhs=xt2sb[:, k, b * BLK + 1:(b + 1) * BLK + 1],
                        start=False, stop=(k == KT2 - 1))
                o2q = tmp.tile([128, BLK], fp32, tag="o2q", name=f"o2q_{b}_{q}")
                nc.scalar.activation(
                    out=o2q, in_=ps,
                    func=Act.Identity, bias=b2sb[:, q, :], scale=1.0)
                for c in range(4):
                    pt = pstr.tile([128, 128], fp32, tag="pt", name=f"pt_{b}_{q}_{c}")
                    nc.tensor.transpose(pt, o2q[:, c * 128:(c + 1) * 128], ident)
                    nc.vector.tensor_copy(
                        out=rowcs[c][:, q * 128:(q + 1) * 128], in_=pt)
                if q % 4 == 3:
                    # column group c4 = q//4 (cols c4*512 .. +512) complete for
                    # every chunk: fold residual + partial sum-of-squares now so
                    # almost no norm work remains after the last matmul.
                    c4 = q // 4
                    sl = slice(c4 * BLK, (c4 + 1) * BLK)
                    if c4 < 3:
                        load_xr_group(c4 + 1)
                    for c in range(4):
                        nc.vector.tensor_add(
                            out=rowcs[c][:, sl], in0=rowcs[c][:, sl],
                            in1=xrcs[(c, c4)])
                        dump = psdp.tile([128, BLK], fp32, tag="dump",
                                         name=f"dump_{b}_{c}_{c4}")
                        nc.scalar.activation(
                            out=dump, in_=rowcs[c][:, sl],
                            func=Act.Square, accum_out=accs[c][:, c4:c4 + 1])
            # finalize per 128-token chunk: rstd + scale + store
            for c in range(4):
                tok0 = b * BLK + c * 128
                rstd = tmp.tile([128, 1], fp32, tag="rstd", name=f"rstd_{b}_{c}")
                nc.vector.tensor_reduce(
                    out=rstd, in_=accs[c], axis=mybir.AxisListType.X,
                    op=mybir.AluOpType.add)
                nc.scalar.activation(
                    out=rstd, in_=rstd,
                    func=Act.Sqrt, bias=epssb, scale=1.0 / D)
                nc.vector.reciprocal(out=rstd, in_=rstd)
                if c % 2 == 0:
                    nc.scalar.activation(
                        out=rowcs[c], in_=rowcs[c],
                        func=Act.Identity, bias=0.0, scale=rstd)
                else:
                    nc.vector.tensor_scalar_mul(
                        out=rowcs[c], in0=rowcs[c], scalar1=rstd)
                nc.sync.dma_start(out=out[tok0:tok0 + 128, :], in_=rowcs[c])

    nc.finalize()
    _NC_CACHE["nc"] = nc
    return nc


def _np_reference(inputs, pre_lf_indexs, out_lf_indexs, input_lf_loc, out_lf_loc,
                  inputs_loc, outputs_loc, lf1_caches, lf2_caches,
                  conv1_weight, conv2_weight, conv1_bias, conv2_bias, ln_weight):
    """Generic numpy fallback (only used if the index structure is unexpected)."""
    def fused(x, cache, pre_idx, in_lf_loc, in_loc, out_loc, W):
        bs = pre_idx.shape[0]
        xt = np.zeros((x.shape[0] + bs, x.shape[1]), x.dtype)
        xt[in_loc] = x
        xt[in_lf_loc] = cache[pre_idx]
        c = xt @ W
        h = c.shape[1] // 2
        y = c[:-1, :h] + c[1:, h:]
        return y[out_loc]

    o1 = fused(inputs, lf1_caches, pre_lf_indexs, input_lf_loc,
               inputs_loc, outputs_loc, conv1_weight) + conv1_bias
    o2 = fused(o1, lf2_caches, pre_lf_indexs, input_lf_loc,
               inputs_loc, outputs_loc, conv2_weight) + conv2_bias
    o3 = o2 + inputs
    var = np.mean(o3 * o3, axis=-1, keepdims=True)
    return (o3 / np.sqrt(var + EPS) * ln_weight).astype(np.float32)


def kernel(**inputs):
    global LAST_EXEC_NS, LAST_RESULTS
    inp = {k: np.asarray(v) for k, v in inputs.items()}
    x = inp["inputs"].astype(np.float32, copy=False)
    lnw = inp["ln_weight"].astype(np.float32, copy=False)

    s = np.arange(BS, dtype=np.int64)
    j = np.arange(L, dtype=np.int64)
    structured = (
        np.array_equal(inp["inputs_loc"], (s[:, None] * (L + 1) + 1 + j[None, :]).reshape(-1))
        and np.array_equal(inp["outputs_loc"], (s[:, None] * (L + 1) + j[None, :]).reshape(-1))
        and np.array_equal(inp["input_lf_loc"], s * (L + 1))
    )
    if not structured:
        return _np_reference(**inp)

    from concourse.bass_utils import run_bass_kernel_spmd

    nc = _build_bass()

    bf16 = ml_dtypes.bfloat16
    pre_idx = inp["pre_lf_indexs"].astype(np.int64)
    w1b = np.ascontiguousarray(inp["conv1_weight"].astype(bf16))
    w2b = np.ascontiguousarray(inp["conv2_weight"].astype(bf16))
    b1f = np.ascontiguousarray(inp["conv1_bias"].astype(np.float32).reshape(H, 1))
    b2f = np.ascontiguousarray(inp["conv2_bias"].astype(np.float32).reshape(D, 1))

    in_maps = []
    for sq in range(BS):
        xs = x[sq * L:(sq + 1) * L]                       # [2048, 2048]
        a = np.empty((D, L + 1), np.float32)
        a[:, 0] = inp["lf1_caches"][pre_idx[sq]]
        a[:, 1:] = xs.T
        in_maps.append({
            "xt1": np.ascontiguousarray(a.astype(bf16)),
            "xrow": np.ascontiguousarray(xs),
            "c2": np.ascontiguousarray(
                inp["lf2_caches"][pre_idx[sq]].astype(bf16).reshape(H, 1)),
            "w1": w1b,
            "w2": w2b,
            "b1": b1f,
            "b2": b2f,
        })

    res = run_bass_kernel_spmd(nc, in_maps, list(range(NCORES)), trace=TRACE)
    LAST_EXEC_NS = res.exec_time_ns
    LAST_RESULTS = res
    out = np.concatenate([res.results[i]["out"] for i in range(NCORES)], axis=0)
    if not np.all(lnw == 1.0):
        out = out * lnw[None, :]
    return out.astype(np.float32)



# revision 10
# speedup vs baseline: 1.1006x; 1.1006x over previous
"""Trainium2 Bass kernel for nn_LocalizedFiltering (fused cat-conv2d x2 + residual + RMSNorm).

Strategy: sequence-parallel across 8 NeuronCores (one sequence of 2048 tokens +
1 cache row per core) -- no collectives needed. Matmuls run in bf16 (fp32 PSUM
accumulation); residual + RMSNorm in fp32.

Layout plan (keeps the PE array 100% on matmuls -- no on-chip transposes):
  Phase A (layer 1), feature-major: psum[feat, tok] = sum_k W1_k^T @ xT windows.
    Output features land on partitions -> per-partition bias add via the
    activation engine while copying psum -> xt2 (bf16), which is exactly the
    feature-major (lhsT) layout phase B needs.
  Phase B (layer 2), token-major: psum[tok, feat] = sum_k xt2_k^T @ W2 windows.
    The kernel-2 causal shift becomes a +-1 column offset of the xt2 lhsT
    window. Tokens land on partitions, so residual + bias (vector add with
    host-precomputed xres = x + b2) and RMSNorm (per-partition rstd) follow
    directly, and rows DMA straight out -- no transposes anywhere.
ln_weight is applied exactly on the host (out *= ln_weight).
"""

import os

import numpy as np
import ml_dtypes

BS, L, D, CACHE = 8, 2048, 2048, 64
T = BS * L
H = D // 2          # 1024
EPS = 1e-6
NCORES = 8
BLK = 512           # token block (= one PSUM bank of fp32)
NBLK = L // BLK     # 4
KT1 = D // 128      # 16 contraction tiles, layer 1
KT2 = H // 128      # 8 contraction tiles, layer 2
QT1 = H // 128      # 8 output-feature tiles, layer 1 (per half)
NTT = L // 128      # 16 token tiles, layer 2
FS = 512            # feature slice, layer 2 output
NFS = D // FS       # 4

TRACE = bool(int(os.environ.get("BASS_KERNEL_TRACE", "0")))
LAST_EXEC_NS = None
LAST_RESULTS = None

_NC_CACHE = {}


def _build_bass():
    if "nc" in _NC_CACHE:
        return _NC_CACHE["nc"]

    import concourse.bacc as bacc
    import concourse.tile as tile
    import concourse.mybir as mybir

    fp32 = mybir.dt.float32
    bf16 = mybir.dt.bfloat16
    Act = mybir.ActivationFunctionType

    nc = bacc.Bacc("TRN2", target_bir_lowering=False)

    xt1 = nc.declare_dram_parameter("xt1", [D, L + 1], bf16, isOutput=False)
    xres = nc.declare_dram_parameter("xres", [L, D], bf16, isOutput=False)
    c2 = nc.declare_dram_parameter("c2", [H, 1], bf16, isOutput=False)
    w1 = nc.declare_dram_parameter("w1", [D, D], bf16, isOutput=False)
    w2 = nc.declare_dram_parameter("w2", [H, 2 * D], bf16, isOutput=False)
    b1 = nc.declare_dram_parameter("b1", [H, 1], fp32, isOutput=False)
    out = nc.declare_dram_parameter("out", [L, D], bf16, isOutput=True)

    with tile.TileContext(nc) as tc, \
            tc.tile_pool(name="wpool", bufs=1) as wpool, \
            tc.tile_pool(name="wpre", bufs=1) as wpre, \
            tc.tile_pool(name="xt1p", bufs=2) as xt1p, \
            tc.tile_pool(name="xt2p", bufs=1) as xt2p, \
            tc.tile_pool(name="xresp", bufs=2) as xresp, \
            tc.tile_pool(name="rowp", bufs=3) as rowp, \
            tc.tile_pool(name="obp", bufs=2) as obp, \
            tc.tile_pool(name="scr", bufs=2) as scr, \
            tc.tile_pool(name="tmp", bufs=2) as tmp, \
            tc.tile_pool(name="const", bufs=1) as const, \
            tc.tile_pool(name="psp", bufs=8, space="PSUM") as psp:

        epssb = const.tile([128, 1], fp32)
        nc.vector.memset(epssb, EPS)

        # startup: a tiny copy of W1[0:128, 0:128] so the very first matmul
        # waits on ~33KB of DMA, not a full 512KB weight row. Issued on the
        # activation engine's DGE queue so it runs in parallel with the SP
        # queue that carries the weight/x streams.
        wfirst = const.tile([128, 128], bf16, name="wfirst")
        nc.scalar.dma_start(out=wfirst, in_=w1[0:128, 0:128])

        b1sb = const.tile([128, QT1, 1], fp32)
        xt2sb = xt2p.tile([128, KT2, L + 1], bf16)

        # ---------------- Phase A: layer 1 -> xt2 (bf16, feature-major) -----
        # W1 as 8 pair-tiles [128, 2, D]; the same slots are later reused by
        # the W2 k-tiles. Issue order interleaves weight rows with x tiles so
        # the k-outer matmul stream is never starved at startup.
        NW = KT1 // 2  # 8
        w1t = []
        x1k0 = []

        def w1row_dma(wj, j, kk, split):
            src = w1[(2 * j + kk) * 128:(2 * j + kk + 1) * 128, :]
            if split:
                nc.sync.dma_start(out=wj[:, kk, 0:H], in_=src[:, 0:H])
                nc.sync.dma_start(out=wj[:, kk, H:D], in_=src[:, H:D])
            else:
                nc.sync.dma_start(out=wj[:, kk, :], in_=src)

        for j in range(NW):
            wj = wpool.tile([128, 2, D], bf16, tag=f"w{j}", name=f"w1_{j}")
            w1t.append(wj)
        for k in range(KT1):
            xk = xt1p.tile([128, BLK + 1], bf16, tag=f"x1k{k}", name=f"x1_0_{k}")
            (nc.scalar if k == 0 else nc.sync).dma_start(
                out=xk, in_=xt1[k * 128:(k + 1) * 128, 0:BLK + 1])
            w1row_dma(w1t[k // 2], k // 2, k % 2, split=(k < 2))
            if k == 1:
                nc.scalar.dma_start(
                    out=b1sb, in_=b1.rearrange("(q p) o -> p q o", p=128))
                nc.scalar.dma_start(
                    out=xt2sb[:, :, 0:1], in_=c2.rearrange("(k p) o -> p k o", p=128))
            x1k0.append(xk)

        for b in range(NBLK):
            if b == 0:
                x1k = x1k0
            else:
                x1k = []
                for k in range(KT1):
                    xk = xt1p.tile([128, BLK + 1], bf16, tag=f"x1k{k}",
                                   name=f"x1_{b}_{k}")
                    nc.sync.dma_start(
                        out=xk,
                        in_=xt1[k * 128:(k + 1) * 128, b * BLK:b * BLK + BLK + 1])
                    x1k.append(xk)
            psA = [psp.tile([128, BLK], fp32, tag="mm", name=f"psA_{b}_{q}")
                   for q in range(QT1)]
            # k-outer over 8 concurrent psum banks; the final k round is
            # per-q (matmuls then the act drain) so banks free one by one and
            # the next block / phase B never waits on a bulk drain.
            for k in range(KT1):
                last = (k == KT1 - 1)
                if b == 0 and k == 0:
                    # win-major at the very start: the 8 win-0 matmuls only
                    # need the first half of W1 row 0 (and wfirst), so the PE
                    # starts while the second half is still on the wire.
                    for q in range(QT1):
                        lhs0 = wfirst if q == 0 \
                            else w1t[0][:, 0, q * 128:(q + 1) * 128]
                        nc.tensor.matmul(
                            psA[q], lhsT=lhs0, rhs=x1k[0][:, 0:BLK],
                            start=True, stop=False)
                    for q in range(QT1):
                        nc.tensor.matmul(
                            psA[q], lhsT=w1t[0][:, 0, H + q * 128:H + (q + 1) * 128],
                            rhs=x1k[0][:, 1:BLK + 1],
                            start=False, stop=False)
                    continue
                for q in range(QT1):
                    nc.tensor.matmul(
                        psA[q], lhsT=w1t[k // 2][:, k % 2, q * 128:(q + 1) * 128],
                        rhs=x1k[k][:, 0:BLK],
                        start=(k == 0), stop=False)
                    nc.tensor.matmul(
                        psA[q],
                        lhsT=w1t[k // 2][:, k % 2, H + q * 128:H + (q + 1) * 128],
                        rhs=x1k[k][:, 1:BLK + 1],
                        start=False, stop=last)
                    if last:
                        nc.scalar.activation(
                            out=xt2sb[:, q, 1 + b * BLK:1 + (b + 1) * BLK],
                            in_=psA[q],
                            func=Act.Identity, bias=b1sb[:, q, :], scale=1.0)

        # ---------------- Phase B: layer 2 + residual + RMSNorm -------------
        # token-major: psum[tok, feat]; lhsT = xt2 column windows (the causal
        # shift), rhs = W2 feature slices. W2 k=0,1 in dedicated slots
        # (prefetched during phase A); k>=2 reuse the W1 slots.
        w2t = []
        for k in range(KT2):
            if k < 2:
                wk = wpre.tile([128, 2 * D], bf16, tag=f"wp{k}", name=f"w2_{k}")
            else:
                wk = wpool.tile([128, 2 * D], bf16, tag=f"w{k - 2}", name=f"w2_{k}")
            nc.sync.dma_start(out=wk, in_=w2[k * 128:(k + 1) * 128, :])
            w2t.append(wk)

        for j in range(NTT):
            tok0 = j * 128
            xr = xresp.tile([128, D], bf16, tag="xres", name=f"xres_{j}")
            nc.scalar.dma_start(out=xr, in_=xres[tok0:tok0 + 128, :])
            rowc = rowp.tile([128, D], fp32, tag="rowc", name=f"rowc_{j}")
            ob = obp.tile([128, D], bf16, tag="ob", name=f"ob_{j}")
            acc = tmp.tile([128, NFS], fp32, tag="acc", name=f"acc_{j}")
            for q in range(NFS):
                sl = slice(q * FS, (q + 1) * FS)
                ps = psp.tile([128, FS], fp32, tag="mm", name=f"psB_{j}_{q}")
                for k in range(KT2):
                    nc.tensor.matmul(
                        ps, lhsT=xt2sb[:, k, tok0:tok0 + 128],
                        rhs=w2t[k][:, q * FS:(q + 1) * FS],
                        start=(k == 0), stop=False)
                    nc.tensor.matmul(
                        ps, lhsT=xt2sb[:, k, tok0 + 1:tok0 + 129],
                        rhs=w2t[k][:, D + q * FS:D + (q + 1) * FS],
                        start=False, stop=(k == KT2 - 1))
                # o3 slice = o2 + (x + b2); then partial sum-of-squares so
                # almost no norm work remains after the last matmul.
                nc.vector.tensor_add(out=rowc[:, sl], in0=ps, in1=xr[:, sl])
                sq = scr.tile([128, FS], bf16, tag="sq", name=f"sq_{j}_{q}")
                nc.scalar.activation(
                    out=sq, in_=rowc[:, sl],
                    func=Act.Square, accum_out=acc[:, q:q + 1])
            rstd = tmp.tile([128, 1], fp32, tag="rstd", name=f"rstd_{j}")
            nc.vector.tensor_reduce(
                out=rstd, in_=acc, axis=mybir.AxisListType.X,
                op=mybir.AluOpType.add)
            nc.scalar.activation(
                out=rstd, in_=rstd, func=Act.Sqrt, bias=epssb, scale=1.0 / D)
            nc.vector.reciprocal(out=rstd, in_=rstd)
            for q in range(NFS):
                sl = slice(q * FS, (q + 1) * FS)
                if q % 2 == 0:
                    nc.scalar.activation(
                        out=ob[:, sl], in_=rowc[:, sl],
                        func=Act.Identity, bias=0.0, scale=rstd)
                else:
                    nc.vector.tensor_scalar_mul(
                        out=ob[:, sl], in0=rowc[:, sl], scalar1=rstd)
                nc.scalar.dma_start(
                    out=out[tok0:tok0 + 128, q * FS:(q + 1) * FS],
                    in_=ob[:, sl])

    nc.finalize()
    _NC_CACHE["nc"] = nc
    return nc


def _np_reference(inputs, pre_lf_indexs, out_lf_indexs, input_lf_loc, out_lf_loc,
                  inputs_loc, outputs_loc, lf1_caches, lf2_caches,
                  conv1_weight, conv2_weight, conv1_bias, conv2_bias, ln_weight):
    """Generic numpy fallback (only used if the index structure is unexpected)."""
    def fused(x, cache, pre_idx, in_lf_loc, in_loc, out_loc, W):
        bs = pre_idx.shape[0]
        xt = np.zeros((x.shape[0] + bs, x.shape[1]), x.dtype)
        xt[in_loc] = x
        xt[in_lf_loc] = cache[pre_idx]
        c = xt @ W
        h = c.shape[1] // 2
        y = c[:-1, :h] + c[1:, h:]
        return y[out_loc]

    o1 = fused(inputs, lf1_caches, pre_lf_indexs, input_lf_loc,
               inputs_loc, outputs_loc, conv1_weight) + conv1_bias
    o2 = fused(o1, lf2_caches, pre_lf_indexs, input_lf_loc,
               inputs_loc, outputs_loc, conv2_weight) + conv2_bias
    o3 = o2 + inputs
    var = np.mean(o3 * o3, axis=-1, keepdims=True)
    return (o3 / np.sqrt(var + EPS) * ln_weight).astype(np.float32)


def kernel(**inputs):
    global LAST_EXEC_NS, LAST_RESULTS
    inp = {k: np.asarray(v) for k, v in inputs.items()}
    x = inp["inputs"].astype(np.float32, copy=False)
    lnw = inp["ln_weight"].astype(np.float32, copy=False)

    s = np.arange(BS, dtype=np.int64)
    j = np.arange(L, dtype=np.int64)
    structured = (
        np.array_equal(inp["inputs_loc"], (s[:, None] * (L + 1) + 1 + j[None, :]).reshape(-1))
        and np.array_equal(inp["outputs_loc"], (s[:, None] * (L + 1) + j[None, :]).reshape(-1))
        and np.array_equal(inp["input_lf_loc"], s * (L + 1))
    )
    if not structured:
        return _np_reference(**inp)

    from concourse.bass_utils import run_bass_kernel_spmd

    nc = _build_bass()

    bf16 = ml_dtypes.bfloat16
    pre_idx = inp["pre_lf_indexs"].astype(np.int64)
    w1b = np.ascontiguousarray(inp["conv1_weight"].astype(bf16))
    w2b = np.ascontiguousarray(inp["conv2_weight"].astype(bf16))
    b1f = np.ascontiguousarray(inp["conv1_bias"].astype(np.float32).reshape(H, 1))
    b2f = inp["conv2_bias"].astype(np.float32)

    in_maps = []
    for sq in range(BS):
        xs = x[sq * L:(sq + 1) * L]                       # [2048, 2048]
        a = np.empty((D, L + 1), np.float32)
        a[:, 0] = inp["lf1_caches"][pre_idx[sq]]
        a[:, 1:] = xs.T
        in_maps.append({
            "xt1": np.ascontiguousarray(a.astype(bf16)),
            "xres": np.ascontiguousarray((xs + b2f[None, :]).astype(bf16)),
            "c2": np.ascontiguousarray(
                inp["lf2_caches"][pre_idx[sq]].astype(bf16).reshape(H, 1)),
            "w1": w1b,
            "w2": w2b,
            "b1": b1f,
        })

    res = run_bass_kernel_spmd(nc, in_maps, list(range(NCORES)), trace=TRACE)
    LAST_EXEC_NS = res.exec_time_ns
    LAST_RESULTS = res
    out = np.concatenate(
        [res.results[i]["out"].astype(np.float32) for i in range(NCORES)], axis=0)
    if not np.all(lnw == 1.0):
        out = out * lnw[None, :]
    return out.astype(np.float32)


# revision 14
# speedup vs baseline: 1.1138x; 1.0120x over previous
"""Trainium2 Bass kernel for nn_LocalizedFiltering (fused cat-conv2d x2 + residual + RMSNorm).

Strategy: sequence-parallel across 8 NeuronCores (one sequence of 2048 tokens +
1 cache row per core) -- no collectives needed. Matmuls run in bf16 (fp32 PSUM
accumulation); residual + RMSNorm in fp32.

Layout plan (keeps the PE array 100% on matmuls -- no on-chip transposes):
  Phase A (layer 1), feature-major: psum[feat, tok] = sum_k W1_k^T @ xT windows.
    Output features land on partitions -> per-partition bias add via the
    activation engine while copying psum -> xt2 (bf16), which is exactly the
    feature-major (lhsT) layout phase B needs.
  Phase B (layer 2), token-major: psum[tok, feat] = sum_k xt2_k^T @ W2 windows.
    The kernel-2 causal shift becomes a +-1 column offset of the xt2 lhsT
    window. Tokens land on partitions, so residual + bias (vector add with
    host-precomputed xres = x + b2) and RMSNorm (per-partition rstd) follow
    directly, and rows DMA straight out -- no transposes anywhere.
ln_weight is applied exactly on the host (out *= ln_weight).
"""

import os

import numpy as np
import ml_dtypes

BS, L, D, CACHE = 8, 2048, 2048, 64
T = BS * L
H = D // 2          # 1024
EPS = 1e-6
NCORES = 8
BLK = 512           # token block (= one PSUM bank of fp32)
NBLK = L // BLK     # 4
KT1 = D // 128      # 16 contraction tiles, layer 1
KT2 = H // 128      # 8 contraction tiles, layer 2
QT1 = H // 128      # 8 output-feature tiles, layer 1 (per half)
NTT = L // 128      # 16 token tiles, layer 2
FS = 512            # feature slice, layer 2 output
NFS = D // FS       # 4

TRACE = bool(int(os.environ.get("BASS_KERNEL_TRACE", "0")))
LAST_EXEC_NS = None
LAST_RESULTS = None

_NC_CACHE = {}


def _build_bass():
    if "nc" in _NC_CACHE:
        return _NC_CACHE["nc"]

    import concourse.bacc as bacc
    import concourse.tile as tile
    import concourse.mybir as mybir

    fp32 = mybir.dt.float32
    bf16 = mybir.dt.bfloat16
    Act = mybir.ActivationFunctionType

    nc = bacc.Bacc("TRN2", target_bir_lowering=False)

    xt1 = nc.declare_dram_parameter("xt1", [D, L + 1], bf16, isOutput=False)
    xres = nc.declare_dram_parameter("xres", [L, D], bf16, isOutput=False)
    c2 = nc.declare_dram_parameter("c2", [H, 1], bf16, isOutput=False)
    w1 = nc.declare_dram_parameter("w1", [D, D], bf16, isOutput=False)
    w2 = nc.declare_dram_parameter("w2", [H, 2 * D], bf16, isOutput=False)
    b1 = nc.declare_dram_parameter("b1", [H, 1], fp32, isOutput=False)
    out = nc.declare_dram_parameter("out", [L, D], bf16, isOutput=True)

    with tile.TileContext(nc) as tc, \
            tc.tile_pool(name="wpool", bufs=1) as wpool, \
            tc.tile_pool(name="wpre", bufs=1) as wpre, \
            tc.tile_pool(name="xt1p", bufs=2) as xt1p, \
            tc.tile_pool(name="xt2p", bufs=1) as xt2p, \
            tc.tile_pool(name="xresp", bufs=2) as xresp, \
            tc.tile_pool(name="rowp", bufs=3) as rowp, \
            tc.tile_pool(name="obp", bufs=2) as obp, \
            tc.tile_pool(name="scr", bufs=2) as scr, \
            tc.tile_pool(name="tmp", bufs=2) as tmp, \
            tc.tile_pool(name="const", bufs=1) as const, \
            tc.tile_pool(name="psp", bufs=8, space="PSUM") as psp:

        epssb = const.tile([128, 1], fp32)
        nc.vector.memset(epssb, EPS)

        # startup: a tiny copy of W1[0:128, 0:128] so the very first matmul
        # waits on ~33KB of DMA, not a full 512KB weight row.
        wfirst = const.tile([128, 128], bf16, name="wfirst")
        nc.sync.dma_start(out=wfirst, in_=w1[0:128, 0:128])

        b1sb = const.tile([128, QT1, 1], fp32)
        xt2sb = xt2p.tile([128, KT2, L + 1], bf16)

        # ---------------- Phase A: layer 1 -> xt2 (bf16, feature-major) -----
        # W1 as 8 pair-tiles [128, 2, D]; the same slots are later reused by
        # the W2 k-tiles. Issue order interleaves weight rows with x tiles so
        # the k-outer matmul stream is never starved at startup.
        NW = KT1 // 2  # 8
        w1t = []
        x1k0 = []

        def w1row_dma(wj, j, kk, split):
            src = w1[(2 * j + kk) * 128:(2 * j + kk + 1) * 128, :]
            if split:
                nc.sync.dma_start(out=wj[:, kk, 0:H], in_=src[:, 0:H])
                nc.sync.dma_start(out=wj[:, kk, H:D], in_=src[:, H:D])
            else:
                nc.sync.dma_start(out=wj[:, kk, :], in_=src)

        for j in range(NW):
            wj = wpool.tile([128, 2, D], bf16, tag=f"w{j}", name=f"w1_{j}")
            w1t.append(wj)
        for k in range(KT1):
            xk = xt1p.tile([128, BLK + 1], bf16, tag=f"x1k{k}", name=f"x1_0_{k}")
            nc.sync.dma_start(out=xk, in_=xt1[k * 128:(k + 1) * 128, 0:BLK + 1])
            w1row_dma(w1t[k // 2], k // 2, k % 2, split=(k < 2))
            if k == 1:
                nc.sync.dma_start(
                    out=b1sb, in_=b1.rearrange("(q p) o -> p q o", p=128))
                nc.sync.dma_start(
                    out=xt2sb[:, :, 0:1], in_=c2.rearrange("(k p) o -> p k o", p=128))
            x1k0.append(xk)

        for b in range(NBLK):
            if b == 0:
                x1k = x1k0
            else:
                x1k = []
                for k in range(KT1):
                    xk = xt1p.tile([128, BLK + 1], bf16, tag=f"x1k{k}",
                                   name=f"x1_{b}_{k}")
                    nc.sync.dma_start(
                        out=xk,
                        in_=xt1[k * 128:(k + 1) * 128, b * BLK:b * BLK + BLK + 1])
                    x1k.append(xk)
            psA = [psp.tile([128, BLK], fp32, tag="mm", name=f"psA_{b}_{q}")
                   for q in range(QT1)]
            # k-outer over 8 concurrent psum banks; the final k round is
            # per-q (matmuls then the act drain) so banks free one by one and
            # the next block / phase B never waits on a bulk drain.
            for k in range(KT1):
                last = (k == KT1 - 1)
                if b == 0 and k == 0:
                    # win-major at the very start: the 8 win-0 matmuls only
                    # need the first half of W1 row 0 (and wfirst), so the PE
                    # starts while the second half is still on the wire.
                    for q in range(QT1):
                        lhs0 = wfirst if q == 0 \
                            else w1t[0][:, 0, q * 128:(q + 1) * 128]
                        nc.tensor.matmul(
                            psA[q], lhsT=lhs0, rhs=x1k[0][:, 0:BLK],
                            start=True, stop=False)
                    for q in range(QT1):
                        nc.tensor.matmul(
                            psA[q], lhsT=w1t[0][:, 0, H + q * 128:H + (q + 1) * 128],
                            rhs=x1k[0][:, 1:BLK + 1],
                            start=False, stop=False)
                    continue
                for q in range(QT1):
                    nc.tensor.matmul(
                        psA[q], lhsT=w1t[k // 2][:, k % 2, q * 128:(q + 1) * 128],
                        rhs=x1k[k][:, 0:BLK],
                        start=(k == 0), stop=False)
                    nc.tensor.matmul(
                        psA[q],
                        lhsT=w1t[k // 2][:, k % 2, H + q * 128:H + (q + 1) * 128],
                        rhs=x1k[k][:, 1:BLK + 1],
                        start=False, stop=last)
                    if last:
                        nc.scalar.activation(
                            out=xt2sb[:, q, 1 + b * BLK:1 + (b + 1) * BLK],
                            in_=psA[q],
                            func=Act.Identity, bias=b1sb[:, q, :], scale=1.0)

        # ---------------- Phase B: layer 2 + residual + RMSNorm -------------
        # token-major: psum[tok, feat]; lhsT = xt2 column windows (the causal
        # shift), rhs = W2 feature slices. W2 k=0,1 in dedicated slots
        # (prefetched during phase A); k>=2 reuse the W1 slots.
        w2t = []
        for k in range(KT2):
            if k < 2:
                wk = wpre.tile([128, 2 * D], bf16, tag=f"wp{k}", name=f"w2_{k}")
            else:
                wk = wpool.tile([128, 2 * D], bf16, tag=f"w{k - 2}", name=f"w2_{k}")
            nc.sync.dma_start(out=wk, in_=w2[k * 128:(k + 1) * 128, :])
            w2t.append(wk)

        for j in range(NTT):
            tok0 = j * 128
            xr = xresp.tile([128, D], bf16, tag="xres", name=f"xres_{j}")
            nc.sync.dma_start(out=xr, in_=xres[tok0:tok0 + 128, :])
            rowc = rowp.tile([128, D], fp32, tag="rowc", name=f"rowc_{j}")
            ob = obp.tile([128, D], bf16, tag="ob", name=f"ob_{j}")
            acc = tmp.tile([128, NFS], fp32, tag="acc", name=f"acc_{j}")
            for q in range(NFS):
                sl = slice(q * FS, (q + 1) * FS)
                ps = psp.tile([128, FS], fp32, tag="mm", name=f"psB_{j}_{q}")
                for k in range(KT2):
                    nc.tensor.matmul(
                        ps, lhsT=xt2sb[:, k, tok0:tok0 + 128],
                        rhs=w2t[k][:, q * FS:(q + 1) * FS],
                        start=(k == 0), stop=False)
                    nc.tensor.matmul(
                        ps, lhsT=xt2sb[:, k, tok0 + 1:tok0 + 129],
                        rhs=w2t[k][:, D + q * FS:D + (q + 1) * FS],
                        start=False, stop=(k == KT2 - 1))
                # o3 slice = o2 + (x + b2); then partial sum-of-squares so
                # almost no norm work remains after the last matmul.
                nc.vector.tensor_add(out=rowc[:, sl], in0=ps, in1=xr[:, sl])
                sq = scr.tile([128, FS], bf16, tag="sq", name=f"sq_{j}_{q}")
                nc.scalar.activation(
                    out=sq, in_=rowc[:, sl],
                    func=Act.Square, accum_out=acc[:, q:q + 1])
            rstd = tmp.tile([128, 1], fp32, tag="rstd", name=f"rstd_{j}")
            nc.vector.tensor_reduce(
                out=rstd, in_=acc, axis=mybir.AxisListType.X,
                op=mybir.AluOpType.add)
            nc.scalar.activation(
                out=rstd, in_=rstd, func=Act.Sqrt, bias=epssb, scale=1.0 / D)
            nc.vector.reciprocal(out=rstd, in_=rstd)
            for q in range(NFS):
                sl = slice(q * FS, (q + 1) * FS)
                if q == 0:
                    nc.scalar.activation(
                        out=ob[:, sl], in_=rowc[:, sl],
                        func=Act.Identity, bias=0.0, scale=rstd)
                else:
                    nc.vector.tensor_scalar_mul(
                        out=ob[:, sl], in0=rowc[:, sl], scalar1=rstd)
                if q % 2 == 1:
                    hs = slice((q - 1) * FS, (q + 1) * FS)
                    nc.sync.dma_start(
                        out=out[tok0:tok0 + 128, (q - 1) * FS:(q + 1) * FS],
                        in_=ob[:, hs])

    nc.finalize()
    _NC_CACHE["nc"] = nc
    return nc


def _np_reference(inputs, pre_lf_indexs, out_lf_indexs, input_lf_loc, out_lf_loc,
                  inputs_loc, outputs_loc, lf1_caches, lf2_caches,
                  conv1_weight, conv2_weight, conv1_bias, conv2_bias, ln_weight):
    """Generic numpy fallback (only used if the index structure is unexpected)."""
    def fused(x, cache, pre_idx, in_lf_loc, in_loc, out_loc, W):
        bs = pre_idx.shape[0]
        xt = np.zeros((x.shape[0] + bs, x.shape[1]), x.dtype)
        xt[in_loc] = x
        xt[in_lf_loc] = cache[pre_idx]
        c = xt @ W
        h = c.shape[1] // 2
        y = c[:-1, :h] + c[1:, h:]
        return y[out_loc]

    o1 = fused(inputs, lf1_caches, pre_lf_indexs, input_lf_loc,
               inputs_loc, outputs_loc, conv1_weight) + conv1_bias
    o2 = fused(o1, lf2_caches, pre_lf_indexs, input_lf_loc,
               inputs_loc, outputs_loc, conv2_weight) + conv2_bias
    o3 = o2 + inputs
    var = np.mean(o3 * o3, axis=-1, keepdims=True)
    return (o3 / np.sqrt(var + EPS) * ln_weight).astype(np.float32)


def kernel(**inputs):
    global LAST_EXEC_NS, LAST_RESULTS
    inp = {k: np.asarray(v) for k, v in inputs.items()}
    x = inp["inputs"].astype(np.float32, copy=False)
    lnw = inp["ln_weight"].astype(np.float32, copy=False)

    s = np.arange(BS, dtype=np.int64)
    j = np.arange(L, dtype=np.int64)
    structured = (
        np.array_equal(inp["inputs_loc"], (s[:, None] * (L + 1) + 1 + j[None, :]).reshape(-1))
        and np.array_equal(inp["outputs_loc"], (s[:, None] * (L + 1) + j[None, :]).reshape(-1))
        and np.array_equal(inp["input_lf_loc"], s * (L + 1))
    )
    if not structured:
        return _np_reference(**inp)

    from concourse.bass_utils import run_bass_kernel_spmd

    nc = _build_bass()

    bf16 = ml_dtypes.bfloat16
    pre_idx = inp["pre_lf_indexs"].astype(np.int64)
    w1b = np.ascontiguousarray(inp["conv1_weight"].astype(bf16))
    w2b = np.ascontiguousarray(inp["conv2_weight"].astype(bf16))
    b1f = np.ascontiguousarray(inp["conv1_bias"].astype(np.float32).reshape(H, 1))
    b2f = inp["conv2_bias"].astype(np.float32)

    in_maps = []
    for sq in range(BS):
        xs = x[sq * L:(sq + 1) * L]                       # [2048, 2048]
        a = np.empty((D, L + 1), np.float32)
        a[:, 0] = inp["lf1_caches"][pre_idx[sq]]
        a[:, 1:] = xs.T
        in_maps.append({
            "xt1": np.ascontiguousarray(a.astype(bf16)),
            "xres": np.ascontiguousarray((xs + b2f[None, :]).astype(bf16)),
            "c2": np.ascontiguousarray(
                inp["lf2_caches"][pre_idx[sq]].astype(bf16).reshape(H, 1)),
            "w1": w1b,
            "w2": w2b,
            "b1": b1f,
        })

    res = run_bass_kernel_spmd(nc, in_maps, list(range(NCORES)), trace=TRACE)
    LAST_EXEC_NS = res.exec_time_ns
    LAST_RESULTS = res
    out = np.concatenate(
        [res.results[i]["out"].astype(np.float32) for i in range(NCORES)], axis=0)
    if not np.all(lnw == 1.0):
        out = out * lnw[None, :]
    return out.astype(np.float32)
